# revision 1
# baseline (speedup 1.0000x reference)
"""MultiHeadLocalAttention Trainium2 kernel.

Sharding: data-parallel over batch B=8 across the 8 NeuronCores (one batch
element per core). Each core runs the full pipeline for its element:
QKV projections, banded local attention (window 33 + cls), cls full
attention, and the output projection.

Layouts on-chip (per core):
  xT, QT, KT : [feat(128-part) x 3 tiles, f] with f = abs_token + 16
               (16 zero cols left pad, zero pad right) so every banded
               slice is in-bounds.
  V'_t       : [128, 385] natural layout, rows = abs tokens
               [128t-15, 128t+113), col 384 = ones (for cls row sums).
  OT         : [feat x 3 tiles, 1025] attention output (transposed), col 0
               is the cls token, filled by the cls path.
Scores for a 128-query block use a 161-wide span (160 banded keys + cls
column); softmax has no max-subtraction (scores are ~N(0,1), exp is safe).
"""

import os
import sys

sys.path.insert(0, "/opt/trn_rl_repo")

import numpy as np
from contextlib import ExitStack

import concourse.bass as bass
import concourse.tile as tile
from concourse import bacc, mybir

H, D = 6, 64
WIN, PAD = 33, 16
B, L, E = 8, 1025, 384
NB = 8            # 128-query blocks covering tokens 1..1024
NT = 9            # token tiles
FP = 16           # f = abs + FP for xT/QT/KT
KTW = FP + L + 16         # 1057
XTW = FP + L + 128        # 1169 (V'_8 reads f up to 1153)
SPAN = 160
SW = SPAN + 1             # span + cls col
F32 = mybir.dt.float32
BF = mybir.dt.bfloat16
AF = mybir.ActivationFunctionType
ALU = mybir.AluOpType
SWP = 162                 # padded per-head section stride (even, bf16 align)

TCHUNKS = [(0, 512), (512, 512), (1024, 1)]
YCHUNKS = TCHUNKS


def host_inputs(x_b, Wq, bq, Wk, bk, Wv, bv, Wo, bo):
    """Per-core input dict (numpy). x_b is this core's [L, E] slice."""
    import ml_dtypes
    bf = ml_dtypes.bfloat16
    scale = 1.0 / np.sqrt(np.float32(D))
    wq = np.asarray(Wq, np.float32) * scale
    bq6 = (np.asarray(bq, np.float32) * scale).reshape(6, 64, 1)
    bk6 = np.asarray(bk, np.float32).reshape(6, 64, 1)
    bo_eff = (
        np.asarray(bv, np.float32) @ np.asarray(Wo, np.float32)
        + np.asarray(bo, np.float32)
    ).reshape(1, E)

    # band masks [3, 128, SWP]: variant 0 -> block 0, 1 -> middle, 2 -> block 7
    masks = np.zeros((3, 128, SWP), np.float32)
    r = np.arange(128)[:, None]
    c = np.arange(SPAN)[None, :]
    for v, b in ((0, 0), (1, 3), (2, NB - 1)):
        absk = 128 * b - 15 + c
        m = (c - r >= 0) & (c - r <= 32) & (absk >= 1) & (absk <= L - 1)
        masks[v, :, :SPAN] = m.astype(np.float32)
        masks[v, :, SPAN] = 1.0
    ident = np.eye(128, dtype=np.float32)

    return {
        "x": np.ascontiguousarray(x_b, dtype=bf),
        "wq": np.ascontiguousarray(wq, dtype=bf),
        "wk": np.ascontiguousarray(Wk, dtype=bf),
        "wv": np.ascontiguousarray(Wv, dtype=bf),
        "wo": np.ascontiguousarray(Wo, dtype=bf),
        "bq6": np.ascontiguousarray(bq6),
        "bk6": np.ascontiguousarray(bk6),
        "bo_row": np.ascontiguousarray(bo_eff),
        "masks": np.ascontiguousarray(masks, dtype=bf),
        "ident": np.ascontiguousarray(ident, dtype=bf),
    }


def build_program(nc):
    max_phase = int(os.environ.get("KMAXPHASE", "9"))

    xd = nc.dram_tensor("x", [L, E], BF, kind="ExternalInput").ap()
    wqd = nc.dram_tensor("wq", [E, E], BF, kind="ExternalInput").ap()
    wkd = nc.dram_tensor("wk", [E, E], BF, kind="ExternalInput").ap()
    wvd = nc.dram_tensor("wv", [E, E], BF, kind="ExternalInput").ap()
    wod = nc.dram_tensor("wo", [E, E], BF, kind="ExternalInput").ap()
    bq6d = nc.dram_tensor("bq6", [6, 64, 1], F32, kind="ExternalInput").ap()
    bk6d = nc.dram_tensor("bk6", [6, 64, 1], F32, kind="ExternalInput").ap()
    bord = nc.dram_tensor("bo_row", [1, E], F32, kind="ExternalInput").ap()
    mkd = nc.dram_tensor("masks", [3, 128, SWP], BF, kind="ExternalInput").ap()
    idd = nc.dram_tensor("ident", [128, 128], BF, kind="ExternalInput").ap()
    outd = nc.dram_tensor("out", [L, E], F32, kind="ExternalOutput").ap()

    with tile.TileContext(nc) as tc, ExitStack() as ctx:
        singles = ctx.enter_context(tc.tile_pool(name="singles", bufs=1))
        xpool = ctx.enter_context(tc.tile_pool(name="xnat", bufs=3))
        apool = ctx.enter_context(tc.tile_pool(name="apool", bufs=2))
        atpool = ctx.enter_context(tc.tile_pool(name="atpool", bufs=3))
        spool = ctx.enter_context(tc.tile_pool(name="small", bufs=4))
        ypool = ctx.enter_context(tc.tile_pool(name="ypool", bufs=2))
        ps_big = ctx.enter_context(tc.tile_pool(name="ps_big", bufs=2, space="PSUM"))
        ps_s = ctx.enter_context(tc.tile_pool(name="ps_s", bufs=2, space="PSUM"))
        ps_fix = ctx.enter_context(tc.tile_pool(name="ps_fix", bufs=1, space="PSUM"))
        ps_o = ctx.enter_context(tc.tile_pool(name="ps_o", bufs=2, space="PSUM"))

        def pbig(dt=F32):
            return ps_big.tile([128, 512], dt, tag="pbig", name="pbig")

        def po_tile():
            return ps_o.tile([128, 512], F32, tag="po", name="po")

        # persistent psum scratch (ping-pong) for A-transposes;
        # memset once so the wide copy never reads uninitialized PSUM
        pt_pp = [ps_fix.tile([128, 384], BF, tag=f"ptpp{i}", name=f"ptpp{i}")
                 for i in range(2)]
        for i in range(2):
            nc.vector.memset(pt_pp[i].bitcast(F32)[:], 0.0)

        # ---- persistent SBUF tensors ----
        ident_sb = singles.tile([128, 128], BF, tag="ident", name="ident_sb")
        nc.sync.dma_start(out=ident_sb[:], in_=idd[:])
        ones_sb = singles.tile([1, 128], F32, tag="onesr", name="ones_sb")
        nc.vector.memset(ones_sb[:], 1.0)
        bo_sb = singles.tile([1, E], F32, tag="bo", name="bo_sb")
        nc.sync.dma_start(out=bo_sb[:], in_=bord[:])
        mask_sb = []
        for v in range(3):
            m = singles.tile([128, SWP], BF, tag=f"mask{v}", name=f"mask{v}")
            nc.sync.dma_start(out=m[:], in_=mkd[v])
            mask_sb.append(m)

        wsb = {}
        for nm, dr in (("wq", wqd), ("wk", wkd), ("wv", wvd), ("wo", wod)):
            tiles = []
            for ki in range(3):
                t = singles.tile([128, E], BF, tag=f"{nm}{ki}", name=f"{nm}{ki}")
                nc.sync.dma_start(out=t[:], in_=dr[ki * 128:(ki + 1) * 128, :])
                tiles.append(t)
            wsb[nm] = tiles
        bsb = {}
        for nm, dr in (("bq", bq6d), ("bk", bk6d)):
            tiles = []
            for h in range(6):
                t = singles.tile([64, 1], F32, tag=f"{nm}{h}", name=f"{nm}{h}")
                nc.sync.dma_start(out=t[:], in_=dr[h])
                tiles.append(t)
            bsb[nm] = tiles

        xT = [singles.tile([128, XTW], BF, tag=f"xT{j}", name=f"xT{j}")
              for j in range(3)]
        QT = [singles.tile([64, KTW], BF, tag=f"QT{h}", name=f"QT{h}")
              for h in range(6)]
        KT = [singles.tile([64, KTW], BF, tag=f"KT{h}", name=f"KT{h}")
              for h in range(6)]
        OT = [singles.tile([128, L], BF, tag=f"OT{j}", name=f"OT{j}")
              for j in range(3)]
        Vp = [singles.tile([128, E + 1], BF, tag=f"Vp{t}", name=f"Vp{t}")
              for t in range(NT)]
        vcls_sb = singles.tile([1, E], BF, tag="vcls", name="vcls_sb")

        for j in range(3):
            nc.vector.memset(xT[j][:, 0:FP], 0.0)
            nc.vector.memset(xT[j][:, FP + L:XTW], 0.0)
        for h in range(6):
            nc.vector.memset(KT[h][:, 0:FP], 0.0)
            nc.vector.memset(KT[h][:, FP + L:KTW], 0.0)

        # ---- phase 1: load x, build xT via PE transposes ----
        for t in range(NT):
            rows = min(128, L - t * 128)
            xt = xpool.tile([128, E], BF, tag="xin", name="xt")
            nc.sync.dma_start(out=xt[:rows, :], in_=xd[t * 128:t * 128 + rows, :])
            for j in range(3):
                pt = pbig(BF)
                nc.tensor.transpose(
                    pt[0:128, 0:rows], xt[:rows, j * 128:(j + 1) * 128],
                    ident_sb[0:rows, 0:rows],
                )
                nc.any.tensor_copy(
                    xT[j][:, FP + t * 128: FP + t * 128 + rows], pt[0:128, 0:rows]
                )

        # ---- phase 2: Q/K projections (per-head transposed layout, += bias) ----
        for nm, dest, bias in (("wq", QT, "bq"), ("wk", KT, "bk")) if max_phase >= 2 else ():
            for h in range(6):
                for c0, w in TCHUNKS:
                    pp = pbig()
                    for ki in range(3):
                        nc.tensor.matmul(
                            pp[0:64, 0:w],
                            lhsT=wsb[nm][ki][:, 64 * h:64 * h + 64],
                            rhs=xT[ki][:, FP + c0: FP + c0 + w],
                            start=(ki == 0), stop=(ki == 2),
                        )
                    nc.scalar.activation(
                        out=dest[h][:, FP + c0: FP + c0 + w], in_=pp[0:64, 0:w],
                        func=AF.Identity, bias=bsb[bias][h][:], scale=1.0,
                    )

        # ---- phase 3: V' shifted tiles + ones col; V_cls row ----
        for t in range(NT if max_phase >= 3 else 0):
            pv = pbig()
            for ki in range(3):
                nc.tensor.matmul(
                    pv[:, 0:E],
                    lhsT=xT[ki][:, 128 * t + 1: 128 * t + 129],
                    rhs=wsb["wv"][ki][:, 0:E],
                    start=(ki == 0), stop=(ki == 2),
                )
            nc.any.tensor_copy(Vp[t][:, 0:E], pv[:, 0:E])
            nc.vector.memset(Vp[t][:, E:E + 1], 1.0)
        pvc = po_tile()
        for ki in range(3 if max_phase >= 3 else 0):
            nc.tensor.matmul(
                pvc[0:1, 0:E], lhsT=xT[ki][:, FP:FP + 1], rhs=wsb["wv"][ki][:, 0:E],
                start=(ki == 0), stop=(ki == 2),
            )
        if max_phase >= 3:
            nc.any.tensor_copy(vcls_sb[0:1, :], pvc[0:1, 0:E])

        # ---- phase 4: banded attention blocks ----
        knblk = int(os.environ.get("KNBLK", str(NB)))
        kskip = set(os.environ.get("KSKIP", "").split(","))
        for b in range((knblk if max_phase >= 4 else 0)):
            mv = 0 if b == 0 else (2 if b == NB - 1 else 1)
            for p in range(3):
                ps = ps_s.tile([128, 2 * SWP], F32, tag="ps_s", name="ps")
                for hh in range(2):
                    c0 = hh * SWP
                    h = 2 * p + hh
                    qs = QT[h][0:64, FP + 1 + 128 * b: FP + 129 + 128 * b]
                    nc.tensor.matmul(
                        ps[:, c0:c0 + SPAN], lhsT=qs,
                        rhs=KT[h][0:64, 128 * b + 1: 128 * b + 161],
                        start=True, stop=True,
                    )
                    nc.tensor.matmul(
                        ps[:, c0 + SPAN:c0 + SPAN + 2], lhsT=qs,
                        rhs=KT[h][0:64, FP:FP + 2],
                        start=True, stop=True,
                    )
                a_exp = apool.tile([128, 2 * SWP], BF, tag="a_exp", name="a_exp")
                nc.scalar.activation(out=a_exp[:], in_=ps[:], func=AF.Exp)
                a_m = apool.tile([128, 2 * SWP], BF, tag="a_m", name="a_m")
                sums = spool.tile([128, 2], F32, tag="sums", name="sums")
                for hh in range(2):
                    c0 = hh * SWP
                    nc.vector.scalar_tensor_tensor(
                        out=a_m[:, c0:c0 + SWP], in0=a_exp[:, c0:c0 + SWP],
                        scalar=1.0, in1=mask_sb[mv][:],
                        op0=ALU.mult, op1=ALU.mult,
                        accum_out=sums[:, hh:hh + 1],
                    )
                recips = spool.tile([128, 2], F32, tag="recips", name="recips")
                nc.vector.reciprocal(recips[:], sums[:])
                a_n = apool.tile([128, 2 * SWP], BF, tag="a_n", name="a_n")
                for hh in range(2):
                    c0 = hh * SWP
                    nc.vector.tensor_scalar_mul(
                        a_n[:, c0:c0 + SWP], a_m[:, c0:c0 + SWP],
                        recips[:, hh:hh + 1],
                    )
                po_t = po_tile()
                for hh in range(2):
                    c0 = hh * SWP
                    fo = 128 * p + 64 * hh   # global feature offset of head 2p+hh
                    pt = pt_pp[((b * 3 + p) * 2 + hh) % 2]
                    nc.tensor.transpose(
                        pt[0:128, 0:128], a_n[:, c0:c0 + 128], ident_sb[:]
                    )
                    nc.tensor.transpose(
                        pt[0:32, 128:256], a_n[:, c0 + 128:c0 + SPAN], ident_sb[:]
                    )
                    nc.tensor.transpose(
                        pt[0:1, 256:384], a_n[:, c0 + SPAN:c0 + SPAN + 1],
                        ident_sb[:]
                    )
                    at = atpool.tile([128, 384], BF, tag="at", name="at")
                    nc.vector.tensor_copy(at[:], pt[:, 0:384])
                    nc.tensor.matmul(
                        po_t[64 * hh:64 * hh + 64, 0:128],
                        lhsT=Vp[b][:, fo:fo + 64],
                        rhs=at[:, 0:128], start=True, stop=False,
                    )
                    nc.tensor.matmul(
                        po_t[64 * hh:64 * hh + 64, 0:128],
                        lhsT=Vp[b + 1][0:32, fo:fo + 64],
                        rhs=at[0:32, 128:256], start=False, stop=False,
                    )
                    nc.tensor.matmul(
                        po_t[64 * hh:64 * hh + 64, 0:128],
                        lhsT=vcls_sb[0:1, fo:fo + 64],
                        rhs=at[0:1, 256:384], start=False, stop=True,
                    )
                nc.any.tensor_copy(
                    OT[p][:, 1 + 128 * b: 129 + 128 * b], po_t[:, 0:128]
                )

        # ---- phase 5: cls query (full attention over all keys) ----
        cls_a = singles.tile([128, L], BF, tag="cls_a", name="cls_a")
        cls_b = singles.tile([64, L], BF, tag="cls_b", name="cls_b")
        acls = singles.tile([6, FP - 1 + L + 129], BF, tag="acls", name="acls")
        nc.vector.memset(acls[:, 0:FP - 1], 0.0)
        nc.vector.memset(acls[:, FP - 1 + L:], 0.0)
        for c0, w in (YCHUNKS if max_phase >= 5 else ()):
            pa = pbig()
            pb = pbig()
            nc.vector.memset(pa[:], 0.0)
            nc.vector.memset(pb[:], 0.0)
            for h in range(6):
                dst = pa if h < 4 else pb
                base = 32 * (h % 4)
                nc.tensor.matmul(
                    dst[base:base + 1, 0:w],
                    lhsT=QT[h][0:64, FP:FP + 1],
                    rhs=KT[h][0:64, FP + c0:FP + c0 + w],
                    start=True, stop=True,
                    tile_position=(0, base),
                )
            nc.scalar.activation(out=cls_a[:, c0:c0 + w], in_=pa[:, 0:w], func=AF.Exp)
            nc.scalar.activation(out=cls_b[:, c0:c0 + w], in_=pb[0:64, 0:w],
                                 func=AF.Exp)
        for h in range(6 if max_phase >= 5 else 0):
            src = cls_a if h < 4 else cls_b
            nc.sync.dma_start(
                out=acls[h:h + 1, FP - 1:FP - 1 + L],
                in_=src[32 * (h % 4):32 * (h % 4) + 1, :],
            )
        aclsT = singles.tile([128, 6 * NT], BF, tag="aclsT", name="aclsT")
        for t in range(NT if max_phase >= 5 else 0):
            pt = pbig(BF)
            nc.tensor.transpose(
                pt[0:128, 0:6], acls[0:6, 128 * t:128 * t + 128],
                ident_sb[0:6, 0:6],
            )
            nc.any.tensor_copy(aclsT[:, 6 * t:6 * t + 6], pt[0:128, 0:6])
        poc = po_tile()
        for t in range(NT if max_phase >= 5 else 0):
            nc.tensor.matmul(
                poc[0:6, 0:E + 1], lhsT=aclsT[:, 6 * t:6 * t + 6], rhs=Vp[t][:],
                start=(t == 0), stop=(t == NT - 1),
            )
        if max_phase >= 5:
            rc = spool.tile([6, 1], F32, tag="rcls", name="rc")
            nc.vector.reciprocal(rc[:], poc[0:6, E:E + 1])
            ocls = singles.tile([6, E], BF, tag="ocls", name="ocls")
            nc.vector.tensor_scalar_mul(ocls[:], poc[0:6, 0:E], rc[:])
            for h in range(6):
                p, po = h // 2, 64 * (h % 2)
                nc.sync.dma_start(
                    out=OT[p][po:po + 64, 0:1],
                    in_=ocls[h:h + 1, 128 * p + po:128 * p + po + 64],
                )
        else:
            for p in range(3):
                nc.vector.memset(OT[p][:, 0:1], 0.0)
            if max_phase < 4:
                for p in range(3):
                    nc.vector.memset(OT[p][:, 1:L], 0.0)

        # ---- phase 6: output projection y = O @ Wo + bo_eff ----
        for t in range(NT if max_phase >= 6 else 0):
            rows = min(128, L - t * 128)
            py = pbig()
            for ki in range(3):
                nc.tensor.matmul(
                    py[0:rows, 0:E],
                    lhsT=OT[ki][:, 128 * t:128 * t + rows],
                    rhs=wsb["wo"][ki][:, 0:E],
                    start=(ki == 0), stop=False,
                )
            nc.tensor.matmul(
                py[0:rows, 0:E], lhsT=ones_sb[0:1, 0:rows], rhs=bo_sb[:],
                start=False, stop=True,
            )
            ysb = ypool.tile([128, E], F32, tag="ysb", name="ysb")
            nc.any.tensor_copy(ysb[0:rows, :], py[0:rows, 0:E])
            nc.sync.dma_start(out=outd[128 * t:128 * t + rows, :], in_=ysb[0:rows, :])

    nc.compile()
    return nc


_CACHE = {}


def get_nc():
    if "nc" not in _CACHE:
        nc = bacc.Bacc("TRN2", target_bir_lowering=False, debug=False)
        _CACHE["nc"] = build_program(nc)
    return _CACHE["nc"]


def kernel(x, Wq, bq, Wk, bk, Wv, bv, Wo, bo, _trace=False):
    from concourse.bass_utils import run_bass_kernel_spmd

    x = np.asarray(x)
    in_maps = [
        host_inputs(x[b], Wq, bq, Wk, bk, Wv, bv, Wo, bo) for b in range(B)
    ]
    nc = get_nc()
    res = run_bass_kernel_spmd(nc, in_maps, core_ids=list(range(8)), trace=_trace)
    out = np.stack([res.results[b]["out"] for b in range(B)], axis=0)
    if _trace:
        return out, res
    return out

